# revision 31
# baseline (speedup 1.0000x reference)
"""Masked self-attention Trainium2 kernel (8 NeuronCores, Bass/Tile).

Problem: B=4, S=2048, D=1024, DK=128 fp32.
  Q = X@Wq + bq; K = X@Wk + bk; V = X@Wv + bv
  scores = Q@K^T / sqrt(DK); masked = scores + tril(ones)*(-1e9)
  out = softmax(masked) @ V

Sharding: core = (batch b = core//2) x (row-half h = core%2). Each core
computes 64 query rows of each of the 16 query tiles of its batch
(rows 128c + 64h + j). All cores run an identical program; per-core
differences are carried entirely in the input data (a column
permutation of X^T and a small mask block).

Device layouts (all transposed so the PE contracts over partitions):
  X^T [D, S] (host-transposed, per-tile column permuted: own rows first)
  Q^T/K^T [DK, *] = W-chunks(lhsT) x X^T(moving) fp16 matmuls
  scores^T [s-chunk 128, q-prefix] = K^T-chunk(lhsT) x Q^T(moving)
  causal skip: chunk c only attends query tiles qi <= c -> contiguous
  q-prefix of width 64*(c+1); single [128,64] mask block on the last
  64 columns (the diagonal tile)
  softmax: exp without max-subtraction (scores are O(1); masked lanes
  underflow to exactly 0); row sums via an M=1 all-ones matmul.

The device returns UNNORMALIZED out^T [DK, 1024] plus the row sums
[1, 1024]; the softmax division, the V bias, and the globally
fully-masked last row (2047 = mean of V) are applied on the host.

Scheduling (the engine queues execute strictly in emission order, and
the PE stalls whenever the next emitted matmul's inputs aren't ready):
  - warm-up matmuls on zeroed scratch ramp the PE clock during the
    initial DMA wait (the clock starts ~1.2GHz and needs ~3us of
    continuous execution to reach 2.4GHz; idle gaps reset the ramp)
  - projection groups of block b+1 are interleaved between the
    attention pieces of block b, so the PE has independent work while
    the score->mask->exp->PV dependency chain of each piece drains
    (a group's matmuls must stay contiguous on the PE queue)
  - exp is split: non-diagonal columns don't wait for the DVE mask add
  - block 3 has no next-block projections, so its 8 attention pieces
    run as a depth-3 software pipeline (score matmuls 3 pieces ahead
    of the PV/sums matmuls, 2 extra PSUM score buffers borrowed from
    the idle projection pool)
  - readers of an open PSUM accumulation group get no dependency edge,
    so the final matmul writing each output range carries stop=True
  - output columns complete progressively (block-3 chunks descending)
    and are copied/DMA'd out while the remaining chunks compute
"""

import numpy as np

import concourse.bacc as bacc
import concourse.tile as tile
import concourse.mybir as mybir
from concourse.bass_utils import run_bass_kernel_spmd

F32 = mybir.dt.float32
F16 = mybir.dt.float16
AF = mybir.ActivationFunctionType

B, S, D, DK = 4, 2048, 1024, 128
NEG = -1.0e9
NCORES = 8
NBLK = 4          # s-blocks of 512
NCHUNK = 16       # s-chunks of 128
QL = 1024         # local query columns per core (16 tiles x 64)
NWARM = 6         # warm-up matmuls (512 cols each) to ramp the PE clock

_cache = {}


def _build():
    nc = bacc.Bacc("TRN2", target_bir_lowering=False, debug=False,
                   num_devices=NCORES)

    # X^T tiled [blk, p, dc, s]: every DMA reads contiguous 2KB runs
    # per partition (1KB strided lines run the DMA engines at ~50%)
    xt = nc.dram_tensor("xt", [NBLK, 128, 8, 512], F16, kind="ExternalInput")
    wq = nc.dram_tensor("wq", [128, 8, DK], F16, kind="ExternalInput")
    wk = nc.dram_tensor("wk", [128, 8, DK], F16, kind="ExternalInput")
    wv = nc.dram_tensor("wv", [128, 8, DK], F16, kind="ExternalInput")
    bqk = nc.dram_tensor("bqk", [DK, 2], F32, kind="ExternalInput")
    maskd = nc.dram_tensor("maskd", [128, 64], F32, kind="ExternalInput")
    idend = nc.dram_tensor("idend", [128, 128], F16, kind="ExternalInput")
    outT = nc.dram_tensor("outT", [DK, QL], F32, kind="ExternalOutput")
    sumsd = nc.dram_tensor("sums", [1, QL], F32, kind="ExternalOutput")

    with tile.TileContext(nc) as tc:
        with (
            tc.tile_pool(name="consts", bufs=1) as cpool,
            tc.tile_pool(name="xblk", bufs=3) as xpool,
            tc.tile_pool(name="kv", bufs=1) as kvpool,
            tc.tile_pool(name="pt", bufs=4) as ppool,
            tc.tile_pool(name="outp", bufs=1) as opool,
            tc.tile_pool(name="ps_out", bufs=1, space="PSUM") as ps_out_pool,
            tc.tile_pool(name="ps_sums", bufs=1, space="PSUM") as ps_sums_pool,
            tc.tile_pool(name="ps_proj", bufs=2, space="PSUM") as ps_proj_pool,
            tc.tile_pool(name="ps_score", bufs=2, space="PSUM") as ps_score_pool,
        ):
            # ---- first wave: the DMAs that gate the first real matmul.
            # X on the sync queue, weights on scalar, consts on gpsimd; the
            # tensor queue carries only LDWEIGHTS/matmul so it is never
            # blocked behind ~700ns DMA descriptor generation.
            w_sb = {}
            for name in ("k", "v", "q"):
                t = cpool.tile([128, 8, DK], F16, tag=f"w{name}")
                w_sb[name] = t
            nc.scalar.dma_start(out=w_sb["k"][:, 0:1], in_=wk[:, 0:1])

            # X^T blocks arrive as four per-dc-pair tiles so each projection
            # matmul gates on its own 256KiB slice; block 0's first tile is
            # further split in half so the very first matmul starts sooner.
            def x_tiles(blk):
                ts = []
                for i in range(4):
                    t = xpool.tile([128, 2, 512], F16, tag=f"xb{i}")
                    src = xt[blk, :, 2 * i:2 * i + 2, :]
                    if blk == 0 and i == 0:
                        nc.sync.dma_start(out=t[:, 0:1], in_=src[:, 0:1])
                        nc.sync.dma_start(out=t[:, 1:2], in_=src[:, 1:2])
                    else:
                        nc.sync.dma_start(out=t[:], in_=src)
                    ts.append(t)
                return ts

            xb_cur = x_tiles(0)

            # ---- PE warm-up on zeroed scratch (no data dependencies).
            warm_w = cpool.tile([128, 128], F16, tag="warmw")
            warm_x = cpool.tile([128, 512], F16, tag="warmx")
            nc.gpsimd.memset(warm_w[:], 0.0)
            nc.gpsimd.memset(warm_x[:], 0.0)
            ps_warm = ps_score_pool.tile([128, 512], F32, tag="sc")
            for i in range(NWARM):
                nc.tensor.matmul(ps_warm[:], warm_w[:], warm_x[:],
                                 start=(i == 0), stop=(i == NWARM - 1))

            # ---- remaining weight / const DMAs and accumulator memsets.
            nc.scalar.dma_start(out=w_sb["k"][:, 1:8], in_=wk[:, 1:8])
            nc.scalar.dma_start(out=w_sb["v"][:], in_=wv[:])
            nc.scalar.dma_start(out=w_sb["q"][:], in_=wq[:])

            b_sb = cpool.tile([DK, 2], F32, tag="bqk")
            nc.gpsimd.dma_start(out=b_sb[:], in_=bqk[:])
            mask_sb = cpool.tile([128, 64], F32, tag="mask")
            nc.gpsimd.dma_start(out=mask_sb[:], in_=maskd[:])
            iden_sb = cpool.tile([128, 128], F16, tag="iden")
            nc.gpsimd.dma_start(out=iden_sb[:], in_=idend[:])
            ones_sb = cpool.tile([128, 1], F16, tag="ones")
            nc.gpsimd.memset(ones_sb[:], 1.0)

            # ---- persistent buffers ----
            kT_sb = kvpool.tile([DK, S], F16, tag="kT")
            qT_sb = kvpool.tile([DK, QL], F16, tag="qT")
            vT_sb = kvpool.tile([DK, S], F16, tag="vT")
            vnat_sb = kvpool.tile([128, NCHUNK, DK], F16, tag="vnat")

            ps_out = ps_out_pool.tile([DK, QL], F32)       # 2 banks
            ps_sums = ps_sums_pool.tile([1, QL], F32)      # 2 banks
            nc.vector.memset(ps_out[:], 0.0)
            nc.vector.memset(ps_sums[:], 0.0)

            o_sb = opool.tile([DK, QL], F32, tag="o")
            s_sb = opool.tile([1, QL], F32, tag="s")

            def gen_proj(blk, xb):
                """Projection work for block blk; yields after each complete
                accumulation group (a group's matmuls must stay contiguous
                on the PE queue) so attention emission can interleave."""
                s0 = blk * 512
                for name, dst in (("k", kT_sb), ("v", vT_sb)):
                    pp = ps_proj_pool.tile([DK, 512], F32, tag="pp")
                    for dc in range(8):
                        nc.tensor.matmul(
                            pp[:], w_sb[name][:, dc], xb[dc // 2][:, dc % 2],
                            start=(dc == 0), stop=(dc == 7),
                        )
                    if name == "k":
                        nc.vector.tensor_scalar_add(
                            dst[:, s0:s0 + 512], pp[:], b_sb[:, 1:2])
                    else:
                        nc.vector.tensor_copy(dst[:, s0:s0 + 512], pp[:])
                    yield
                pq = ps_proj_pool.tile([DK, 256], F32, tag="pp")
                for dc in range(8):
                    qmov = (xb[dc // 2][:, dc % 2]
                            .rearrange("p (t j) -> p t j", t=4)[:, :, 0:64])
                    nc.tensor.matmul(
                        pq[:], w_sb["q"][:, dc], qmov,
                        start=(dc == 0), stop=(dc == 7),
                    )
                q0 = blk * 256
                nc.vector.tensor_scalar_add(qT_sb[:, q0:q0 + 256], pq[:],
                                            b_sb[:, 0:1])
                yield
                tp4 = ps_proj_pool.tile([128, 4, 128], F16, tag="pp")
                for t in range(4):
                    c = 4 * blk + t
                    nc.tensor.matmul(
                        tp4[:, t], vT_sb[:, 128 * c:128 * c + 128], iden_sb[:],
                        is_transpose=True, start=(t == 0), stop=(t == 3),
                    )
                nc.vector.tensor_copy(vnat_sb[:, 4 * blk:4 * blk + 4], tp4[:])
                yield

            def emit_sc(c, p0, pn, borrow):
                """Score matmul + mask + exp for piece (p0,pn) of chunk c.
                Returns the pt tile. The diagonal (last 64 cols of the
                prefix) exps separately so the bulk doesn't wait for the
                DVE mask add."""
                prefix = 64 * (c + 1)
                pool = ps_proj_pool if borrow else ps_score_pool
                tag = "pp" if borrow else "sc"
                sc = pool.tile([128, 512], F32, tag=tag)
                nc.tensor.matmul(
                    sc[:, 0:pn], kT_sb[:, 128 * c:128 * c + 128],
                    qT_sb[:, p0:p0 + pn], start=True, stop=True,
                )
                pt = ppool.tile([128, 512], F16, tag="pt")
                has_diag = p0 + pn == prefix
                cut = pn - 64 if has_diag else pn
                if cut > 0:
                    nc.scalar.activation(pt[:, 0:cut], sc[:, 0:cut], AF.Exp)
                if has_diag:
                    nc.vector.tensor_tensor(
                        sc[:, cut:pn], sc[:, cut:pn], mask_sb[:],
                        mybir.AluOpType.add,
                    )
                    nc.scalar.activation(pt[:, cut:pn], sc[:, cut:pn], AF.Exp)
                return pt

            def emit_outsums(c, p0, pn, pt, stop_from=None):
                """PV + row-sum matmuls for a piece. Columns >= stop_from
                get stop=True on their final matmul (readers of an open
                PSUM accumulation group get no dependency edge)."""
                if stop_from is None or p0 + pn <= stop_from:
                    subs = [(0, pn, False)]
                elif p0 >= stop_from:
                    subs = [(0, pn, True)]
                else:
                    cut = stop_from - p0
                    subs = [(0, cut, False), (cut, pn, True)]
                for dst, lhs in ((ps_out, vnat_sb[:, c]),
                                 (ps_sums, ones_sb[:])):
                    for (a, b2, stop) in subs:
                        nc.tensor.matmul(
                            dst[:, p0 + a:p0 + b2], lhs,
                            pt[:, a:b2], start=False, stop=stop,
                        )

            def gen_attn(blk):
                """Attention for blocks 0-2: per-piece sequential emission
                (interleaved projections provide latency-hiding work)."""
                for t in range(4):
                    c = 4 * blk + t
                    prefix = 64 * (c + 1)
                    for p0 in range(0, prefix, 512):
                        pn = min(512, prefix - p0)
                        pt = emit_sc(c, p0, pn, borrow=False)
                        emit_outsums(c, p0, pn, pt)
                        yield

            def drive(agen, pgen, n_attn, n_proj):
                """Alternate: one attention piece, then a proportional run
                of projection groups."""
                emitted = 0
                for i, _ in enumerate(agen):
                    want = (i + 1) * n_proj // n_attn
                    while emitted < want:
                        if next(pgen, "end") == "end":
                            emitted = n_proj
                            break
                        emitted += 1
                for _ in pgen:
                    pass

            # ---- block 0 projections: consume per-TILE across all three
            # weights (K/V/Q dc-pairs per 256KiB X tile, each pair a
            # contiguous 2-matmul accumulation sub-group) so PE demand
            # paces the serial X arrival instead of racing ahead of it
            pk = ps_proj_pool.tile([DK, 512], F32, tag="pp")
            pv = ps_proj_pool.tile([DK, 512], F32, tag="pp")
            pq0 = ps_score_pool.tile([128, 512], F32, tag="sc")
            for i in range(4):
                for name, acc in (("k", pk), ("v", pv)):
                    for dc in (2 * i, 2 * i + 1):
                        nc.tensor.matmul(
                            acc[:], w_sb[name][:, dc],
                            xb_cur[i][:, dc % 2],
                            start=(dc == 0), stop=(dc == 2 * i + 1),
                        )
                for dc in (2 * i, 2 * i + 1):
                    qmov = (xb_cur[i][:, dc % 2]
                            .rearrange("p (t j) -> p t j", t=4)[:, :, 0:64])
                    nc.tensor.matmul(
                        pq0[:, 0:256], w_sb["q"][:, dc], qmov,
                        start=(dc == 0), stop=(dc == 2 * i + 1),
                    )
            nc.vector.tensor_scalar_add(kT_sb[:, 0:512], pk[:], b_sb[:, 1:2])
            nc.vector.tensor_copy(vT_sb[:, 0:512], pv[:])
            nc.vector.tensor_scalar_add(qT_sb[:, 0:256], pq0[:, 0:256],
                                        b_sb[:, 0:1])
            tp40 = ps_proj_pool.tile([128, 4, 128], F16, tag="pp")
            for t in range(4):
                nc.tensor.matmul(
                    tp40[:, t], vT_sb[:, 128 * t:128 * t + 128], iden_sb[:],
                    is_transpose=True, start=(t == 0), stop=(t == 3),
                )
            nc.vector.tensor_copy(vnat_sb[:, 0:4], tp40[:])

            # ---- blocks 0-2: attention interleaved with next projections --
            for blk in range(3):
                xb_next = x_tiles(blk + 1)
                n_attn = 4 if blk < 2 else 8
                drive(gen_attn(blk), gen_proj(blk + 1, xb_next), n_attn, 4)

            # ---- block 3: depth-3 software-pipelined attention ----------
            # pieces in descending-chunk order; chunk 12's big piece last
            pieces = [
                (15, 0, 512, None), (15, 512, 512, 960),
                (14, 0, 512, None), (14, 512, 448, 896),
                (13, 0, 512, None), (13, 512, 384, 832),
                (12, 512, 320, 0), (12, 0, 512, 0),
            ]
            # after the final matmul of each range, copy (and DMA) it out
            def store(a, b2, dma_eng=None, dma_rng=None):
                nc.vector.tensor_copy(o_sb[:, a:b2], ps_out[:, a:b2])
                if dma_rng is not None:
                    dma_eng.dma_start(out=outT[:, dma_rng[0]:dma_rng[1]],
                                      in_=o_sb[:, dma_rng[0]:dma_rng[1]])

            stores_after = {
                1: lambda: store(960, 1024),
                3: lambda: store(896, 960),
                5: lambda: store(832, 896, nc.gpsimd, (832, 1024)),
                6: lambda: store(512, 832, nc.gpsimd, (512, 832)),
            }

            def final_stores():
                # split the last copy so the first DMA's descriptor
                # generation overlaps the second copy (different queues)
                nc.vector.tensor_copy(o_sb[:, 256:512], ps_out[:, 256:512])
                nc.gpsimd.dma_start(out=outT[:, 256:512],
                                    in_=o_sb[:, 256:512])
                nc.vector.tensor_copy(o_sb[:, 0:256], ps_out[:, 0:256])
                nc.vector.tensor_copy(s_sb[:], ps_sums[:])
                nc.scalar.dma_start(out=sumsd[:], in_=s_sb[:])
                nc.sync.dma_start(out=outT[:, 0:256], in_=o_sb[:, 0:256])

            LA = 3
            pts = {}
            for i, (c, p0, pn, sf) in enumerate(pieces):
                # the two in-flight-deepest score tiles borrow the (now
                # idle) projection pool's PSUM buffers
                pts[i] = emit_sc(c, p0, pn, borrow=(i % 4 >= 2))
                if i >= LA:
                    j = i - LA
                    (cj, q0, qn, sfj) = pieces[j]
                    emit_outsums(cj, q0, qn, pts.pop(j), stop_from=sfj)
                    if j in stores_after:
                        stores_after[j]()
            for j in range(len(pieces) - LA, len(pieces)):
                (cj, q0, qn, sfj) = pieces[j]
                emit_outsums(cj, q0, qn, pts.pop(j), stop_from=sfj)
                if j in stores_after:
                    stores_after[j]()
            final_stores()

    nc.compile()
    return nc


def _prep_inputs(inputs, Wq, bq, Wk, bk, Wv, bv):
    scale = np.float32(1.0 / np.sqrt(DK))
    wq_s = np.ascontiguousarray((Wq * scale).reshape(8, 128, DK).transpose(1, 0, 2)).astype(np.float16)
    wk_s = np.ascontiguousarray(Wk.reshape(8, 128, DK).transpose(1, 0, 2)).astype(np.float16)
    wv_s = np.ascontiguousarray(Wv.reshape(8, 128, DK).transpose(1, 0, 2)).astype(np.float16)
    bqk = np.stack([np.asarray(bq, dtype=np.float32) * scale,
                    np.asarray(bk, dtype=np.float32)], axis=1)
    bqk = np.ascontiguousarray(bqk, dtype=np.float32)
    iden = np.eye(128, dtype=np.float16)

    p = np.arange(128)[:, None]
    j = np.arange(64)[None, :]
    masks = []
    for h in (0, 1):
        m = np.zeros((128, 64), dtype=np.float32)
        m[(p < 64) & (p <= j)] = NEG
        if h == 1:
            m[p[:, 0] >= 64, :] = NEG
        masks.append(m)

    in_maps = []
    for core in range(NCORES):
        b, h = core // 2, core % 2
        xt = inputs[b].T.reshape(D, 16, 2, 64)
        if h == 1:
            xt = xt[:, :, ::-1, :]
        xt = xt.reshape(8, 128, NBLK, 512).transpose(2, 1, 0, 3)
        xt = np.ascontiguousarray(xt).astype(np.float16)  # [blk, p, dc, s]
        in_maps.append({
            "xt": xt, "wq": wq_s, "wk": wk_s, "wv": wv_s,
            "bqk": bqk, "maskd": masks[h], "idend": iden,
        })
    return in_maps


def kernel(inputs, Wq, bq, Wk, bk, Wv, bv):
    inputs = np.asarray(inputs, dtype=np.float32)
    Wq, bq = np.asarray(Wq), np.asarray(bq)
    Wk, bk = np.asarray(Wk), np.asarray(bk)
    Wv, bv = np.asarray(Wv), np.asarray(bv)
    if "nc" not in _cache:
        _cache["nc"] = _build()
    nc = _cache["nc"]
    in_maps = _prep_inputs(inputs, Wq, bq, Wk, bk, Wv, bv)
    res = run_bass_kernel_spmd(nc, in_maps, list(range(NCORES)))
    out = np.empty((B, S, DK), dtype=np.float32)
    for core in range(NCORES):
        b, h = core // 2, core % 2
        oT = res.results[core]["outT"]          # [DK, 1024] unnormalized
        sums = res.results[core]["sums"]        # [1, 1024]
        with np.errstate(divide="ignore", invalid="ignore"):
            o = (oT / sums).T.reshape(16, 64, DK)   # [c, j, DK]
        out[b].reshape(16, 2, 64, DK)[:, h] = o
    # host-side epilogue: the globally fully-masked last row softmaxes to
    # uniform 1/S -> mean of V; the V bias adds exactly bv after normalize.
    for b in range(B):
        out[b, S - 1, :] = inputs[b].mean(axis=0) @ Wv
    out += bv.astype(np.float32)
    return out
